# revision 25
# baseline (speedup 1.0000x reference)
"""Trainium2 Bass kernel for nn_Attention_layer_41429254537559.

Reference math:
    img_score = einsum('nld,d->nl', img, w)          # [N, L]
    q_score   = einsum('ntd,d->nt', qes, w)          # [N, T]
    logits    = q_score[:,:,None] + img_score[:,None,:]
    att       = softmax(logits, axis=2)              # over L
    out       = qes + einsum('ntl,nld->ntd', att, img)

Key simplification: q_score[n,t] is constant along the softmax axis (L), so it
cancels inside the softmax.  att[n,t,:] == softmax(img_score[n,:]) for every t:
    a[n,:]  = softmax(img @ w)        # [N, L]
    c[n,:]  = a[n,:] @ img[n]         # [N, D]
    out     = qes + c[:,None,:]

Distribution: data-parallel over N across 8 cores (2 batch elements per core).
No collectives needed.

Per-core dataflow (n_loc = 2, L = 196 = 2x98 chunks, D = 1024, T = 32):
  - img loaded as ONE SWDGE cast-DMA per batch element into a [98, 2, 1024]
    bf16 tile (l-rows split into two 98-row chunks across the free dim);
    qes likewise as one [32, 2, 1024] bf16 tile.  SWDGE descriptor emission
    costs ~1us of gpsimd time per dma_start, so fewer/larger DMAs win.
  - w arrives host-replicated as a [128, 1024] bf16 ExternalInput over the
    HWDGE queue — no on-chip broadcast, no cross-engine dependency chain.
  - s[l] = sum_d img[l,d]*w[d]: one DVE affine_mul_reduce per 98-row chunk
    (bf16 in, f32 accumulator column)
  - per n: e = exp(s) on ScalarE (|s| <~ 7, exp safe without max-shift),
    S_n = sum_l e[l] via tiny PE matmuls against a ones column,
    a = e * (1/S_n) (DVE reciprocal + gpsimd partition_broadcast), then
    materialized as [98, 32] bf16 lhsT tiles via tensor_scalar_mul on ones
  - out[n] = qes[n] + a @ img[n] as ONE accumulated PE matmul group per
    512-wide PSUM half: identity@qes first (inputs ready early), then the
    two a32 @ img-chunk matmuls; PSUM accumulates f32
  - PSUM -> SBUF f32 copy per half on alternating engines (ScalarE h0,
    VectorE h1), each half DMA'd out (HWDGE) immediately

8 bf16 warmup matmuls at t=0 keep the PE HAM clock warm; a dummy exp
preloads the ACT exp table during the DMA fill.
"""

import numpy as np

N_CORES = 8
N, L, D, T = 16, 196, 1024, 32
NL = N // N_CORES  # batch elements per core
NC = 2  # l-chunks per batch element
LC = L // NC  # 98 rows per chunk

_CACHE = {}


def _build_nc():
    import concourse.bass as bass
    import concourse.tile as tile
    from concourse import bacc, mybir
    from concourse.masks import make_identity

    f32 = mybir.dt.float32
    bf16 = mybir.dt.bfloat16
    nc = bacc.Bacc(None, target_bir_lowering=False)

    img = nc.dram_tensor("img", [NL, L, D], f32, kind="ExternalInput")
    qes = nc.dram_tensor("qes", [NL, T, D], f32, kind="ExternalInput")
    wb = nc.dram_tensor("wb", [128, D], bf16, kind="ExternalInput")
    out = nc.dram_tensor("out", [NL * T, D], f32, kind="ExternalOutput")

    with tile.TileContext(nc) as tc:
        with (
            tc.tile_pool(name="persist", bufs=1) as pp,
            tc.tile_pool(name="scratch", bufs=2) as sp,
            tc.tile_pool(name="psum", bufs=1, space="PSUM") as psp,
        ):
            # ---- persistent SBUF tiles ----
            w_b = pp.tile([128, D], bf16, tag="w_b")
            img_t = [pp.tile([LC, NC, D], bf16, tag=f"img{n}", name=f"img{n}") for n in range(NL)]
            qes_t = pp.tile([T, NL, D], bf16, tag="qes_t")
            out_sb = pp.tile([NL * T, D], f32, tag="out_sb")
            s_all = pp.tile([LC, NC * NL], f32, tag="s_all")
            e_all = pp.tile([LC, NC * NL], f32, tag="e_all")
            ones32 = pp.tile([LC, T], f32, tag="ones32")
            ones_col = pp.tile([LC, 1], f32, tag="ones_col")
            eye32 = pp.tile([T, T], bf16, tag="eye32")
            s_sb = pp.tile([1, NL], f32, tag="s_sb")
            warm = pp.tile([128, 512], bf16, tag="warm")
            dummy = pp.tile([1, 1], f32, tag="dummy")
            dummy_o = pp.tile([1, 1], f32, tag="dummy_o")

            # ---- PSUM tiles (6 banks: 1 + 1 + 2*2) ----
            ps_warm = psp.tile([128, 512], f32, tag="ps_warm")
            ps_s = psp.tile([1, NL], f32, tag="ps_s")
            ps_out = [psp.tile([T, D], f32, tag=f"ps_out{n}", name=f"ps_out{n}") for n in range(NL)]

            # ---- loads ----
            # w host-replicated bf16 over HWDGE (parallel to SWDGE img queue)
            nc.sync.dma_start(out=w_b, in_=wb[:, :])
            # img: one cast-DMA per (batch element, 98-row chunk) so each
            # chunk's score reduce can start as soon as its rows land
            img_src = [
                img[n, :, :].rearrange("(c p) d -> p c d", p=LC) for n in range(NL)
            ]
            for n in range(NL):
                for c in range(NC):
                    nc.gpsimd.dma_start(
                        out=img_t[n][:, c, :], in_=img_src[n][:, c, :]
                    )
            nc.gpsimd.dma_start(out=qes_t, in_=qes[:, :, :].transpose([1, 0, 2]))

            make_identity(nc, eye32)

            # ---- ACT exp-table preload + constants (DVE) ----
            nc.vector.memset(dummy, 0.0)
            nc.scalar.activation(dummy_o, dummy, mybir.ActivationFunctionType.Exp)
            nc.vector.memset(ones_col, 1.0)
            nc.vector.memset(ones32, 1.0)
            nc.vector.memset(warm, 0.0)

            # PE HAM warmup: ~8 bf16 N=512 matmuls ~= 3.4us busy at the cold
            # clock -> HAM flips to 8/8 before the real matmuls arrive.
            for i in range(8):
                nc.tensor.matmul(ps_warm, warm[:, 0:128], warm, start=True, stop=True)

            H = 512
            # ---- per-n pipeline with the S-fold trick ----
            # The output matmuls use UNNORMALIZED weights e = exp(s):
            #     psum = sum_l e[l]*img[l,:] + S_q*qes[t,:]
            # (identity scaled by S_q = bf16-quantized S), and the PSUM->SBUF
            # copy applies 1/S_q.  The qes term is exact (S_q * 1/S_q); the
            # attention term is normalized by S_q instead of S (0.4% on a
            # ~7%-magnitude term).  This takes S entirely off the critical
            # path: each chunk's matmuls fire right after its exp.
            for n in range(NL):
                e32 = []
                for c in range(NC):
                    col = NC * n + c
                    prod = sp.tile([LC, D], bf16, tag="prod", name=f"prod{n}{c}")
                    nc.vector.affine_mul_reduce(
                        out=prod,
                        accum_out=s_all[:, col : col + 1],
                        in0=img_t[n][:, c, :],
                        in1=w_b[:LC, :],
                        scale=1.0,
                        bias=0.0,
                    )
                    nc.scalar.activation(
                        e_all[:, col : col + 1],
                        s_all[:, col : col + 1],
                        mybir.ActivationFunctionType.Exp,
                    )
                    nc.tensor.matmul(
                        ps_s[0:1, n : n + 1],
                        e_all[:, col : col + 1],
                        ones_col[:, :],
                        start=(c == 0),
                        stop=(c == NC - 1),
                    )
                    # e replicated to [98, 32] bf16 lhsT.  gpsimd normally
                    # (off the DVE critical path); the very last chunk on the
                    # DVE, which is idle by then and closer to the PE handoff.
                    t = sp.tile([LC, T], bf16, tag=f"e32_{c}", name=f"e32_{n}{c}")
                    eng = nc.vector if (n == NL - 1 and c == NC - 1) else nc.gpsimd
                    eng.tensor_scalar_mul(
                        out=t, in0=ones32, scalar1=e_all[:, col : col + 1]
                    )
                    e32.append(t)

                # S path (parallel to the e@img matmuls, not on their chain):
                # S -> bf16-quantized S_q -> eyeS = eye*S_q, recip32 = 1/S_q
                nc.vector.tensor_copy(s_sb[:, n : n + 1], ps_s[0:1, n : n + 1])
                sq_bf = sp.tile([1, 1], bf16, tag="sq_bf", name=f"sq_bf{n}")
                sq_f = sp.tile([1, 1], f32, tag="sq_f", name=f"sq_f{n}")
                nc.gpsimd.tensor_copy(out=sq_bf, in_=s_sb[:, n : n + 1])
                nc.gpsimd.tensor_copy(out=sq_f, in_=sq_bf)
                s32 = sp.tile([T, 1], f32, tag="s32", name=f"s32_{n}")
                nc.gpsimd.partition_broadcast(s32, sq_f)
                eyeS = sp.tile([T, T], bf16, tag="eyeS", name=f"eyeS{n}")
                nc.gpsimd.tensor_scalar_mul(out=eyeS, in0=eye32, scalar1=s32)
                recip32 = sp.tile([T, 1], f32, tag="recip32", name=f"recip32_{n}")
                nc.vector.reciprocal(recip32, s32)

                # e @ img opens each accumulation group; eyeS@qes closes it
                for h in range(0, D, H):
                    for c in range(NC):
                        nc.tensor.matmul(
                            ps_out[n][:, h : h + H],
                            e32[c],
                            img_t[n][:, c, h : h + H],
                            start=(c == 0),
                            stop=False,
                        )
                    nc.tensor.matmul(
                        ps_out[n][:, h : h + H],
                        eyeS,
                        qes_t[:, n, h : h + H],
                        start=False,
                        stop=True,
                    )

                # PSUM -> SBUF with the 1/S_q scale, halves on alternating
                # engines, each DMA'd out immediately
                for h in range(0, D, H):
                    dst = out_sb[n * T : (n + 1) * T, h : h + H]
                    if h == 0 or n < NL - 1:
                        nc.scalar.activation(
                            dst,
                            ps_out[n][:, h : h + H],
                            mybir.ActivationFunctionType.Copy,
                            scale=recip32[:, :],
                        )
                    else:
                        nc.vector.tensor_scalar_mul(
                            out=dst,
                            in0=ps_out[n][:, h : h + H],
                            scalar1=recip32[:, :],
                        )
                    nc.sync.dma_start(
                        out=out[n * T : (n + 1) * T, h : h + H], in_=dst
                    )

    nc.compile()
    return nc


def _make_in_maps(inputs):
    """Shard the full inputs per core (data-parallel over N, 2 each)."""
    import ml_dtypes

    img_features = np.ascontiguousarray(inputs["img_features"], dtype=np.float32)
    qes_features = np.ascontiguousarray(inputs["qes_features"], dtype=np.float32)
    wb = np.ascontiguousarray(
        np.broadcast_to(
            np.asarray(inputs["w"], np.float32).astype(ml_dtypes.bfloat16)[None, :],
            (128, D),
        )
    )
    in_maps = []
    for c in range(N_CORES):
        sl = slice(NL * c, NL * (c + 1))
        in_maps.append({"img": img_features[sl], "qes": qes_features[sl], "wb": wb})
    return in_maps


def kernel(img_features, qes_features, w):
    from concourse.bass_utils import run_bass_kernel_spmd

    if "nc" not in _CACHE:
        _CACHE["nc"] = _build_nc()
    nc = _CACHE["nc"]

    in_maps = _make_in_maps(
        {"img_features": img_features, "qes_features": qes_features, "w": w}
    )
    res = run_bass_kernel_spmd(nc, in_maps, core_ids=list(range(N_CORES)))
    outs = [r["out"].reshape(NL, T, D) for r in res.results]
    return np.concatenate(outs, axis=0)


# revision 26
# speedup vs baseline: 1.0090x; 1.0090x over previous
"""Trainium2 Bass kernel for nn_Attention_layer_41429254537559.

Reference math:
    img_score = einsum('nld,d->nl', img, w)          # [N, L]
    q_score   = einsum('ntd,d->nt', qes, w)          # [N, T]
    logits    = q_score[:,:,None] + img_score[:,None,:]
    att       = softmax(logits, axis=2)              # over L
    out       = qes + einsum('ntl,nld->ntd', att, img)

Key simplification: q_score[n,t] is constant along the softmax axis (L), so it
cancels inside the softmax.  att[n,t,:] == softmax(img_score[n,:]) for every t:
    a[n,:]  = softmax(img @ w)        # [N, L]
    c[n,:]  = a[n,:] @ img[n]         # [N, D]
    out     = qes + c[:,None,:]

Distribution: data-parallel over N across 8 cores (2 batch elements per core).
No collectives needed.

Per-core dataflow (n_loc = 2, L = 196 = 2x98 chunks, D = 1024, T = 32):
  - img loaded as ONE SWDGE cast-DMA per batch element into a [98, 2, 1024]
    bf16 tile (l-rows split into two 98-row chunks across the free dim);
    qes likewise as one [32, 2, 1024] bf16 tile.  SWDGE descriptor emission
    costs ~1us of gpsimd time per dma_start, so fewer/larger DMAs win.
  - w arrives host-replicated as a [128, 1024] bf16 ExternalInput over the
    HWDGE queue — no on-chip broadcast, no cross-engine dependency chain.
  - s[l] = sum_d img[l,d]*w[d]: one DVE affine_mul_reduce per 98-row chunk
    (bf16 in, f32 accumulator column)
  - per n: e = exp(s) on ScalarE (|s| <~ 7, exp safe without max-shift),
    S_n = sum_l e[l] via tiny PE matmuls against a ones column,
    a = e * (1/S_n) (DVE reciprocal + gpsimd partition_broadcast), then
    materialized as [98, 32] bf16 lhsT tiles via tensor_scalar_mul on ones
  - out[n] = qes[n] + a @ img[n] as ONE accumulated PE matmul group per
    512-wide PSUM half: identity@qes first (inputs ready early), then the
    two a32 @ img-chunk matmuls; PSUM accumulates f32
  - PSUM -> SBUF f32 copy per half on alternating engines (ScalarE h0,
    VectorE h1), each half DMA'd out (HWDGE) immediately

8 bf16 warmup matmuls at t=0 keep the PE HAM clock warm; a dummy exp
preloads the ACT exp table during the DMA fill.
"""

import numpy as np

N_CORES = 8
N, L, D, T = 16, 196, 1024, 32
NL = N // N_CORES  # batch elements per core
NC = 2  # l-chunks per batch element
LC = L // NC  # 98 rows per chunk

_CACHE = {}


def _build_nc():
    import concourse.bass as bass
    import concourse.tile as tile
    from concourse import bacc, mybir
    from concourse.masks import make_identity

    f32 = mybir.dt.float32
    bf16 = mybir.dt.bfloat16
    nc = bacc.Bacc(None, target_bir_lowering=False)

    img = nc.dram_tensor("img", [NL, L, D], f32, kind="ExternalInput")
    qes = nc.dram_tensor("qes", [NL, T, D], f32, kind="ExternalInput")
    wb = nc.dram_tensor("wb", [128, D], bf16, kind="ExternalInput")
    out = nc.dram_tensor("out", [NL * T, D], f32, kind="ExternalOutput")

    with tile.TileContext(nc) as tc:
        with (
            tc.tile_pool(name="persist", bufs=1) as pp,
            tc.tile_pool(name="scratch", bufs=2) as sp,
            tc.tile_pool(name="psum", bufs=1, space="PSUM") as psp,
        ):
            # ---- persistent SBUF tiles ----
            w_b = pp.tile([128, D], bf16, tag="w_b")
            img_t = [pp.tile([LC, NC, D], bf16, tag=f"img{n}", name=f"img{n}") for n in range(NL)]
            qes_t = pp.tile([T, NL, D], bf16, tag="qes_t")
            out_sb = pp.tile([NL * T, D], f32, tag="out_sb")
            s_all = pp.tile([LC, NC * NL], f32, tag="s_all")
            e_bf = pp.tile([LC, NC * NL], bf16, tag="e_bf")
            ones_col = pp.tile([LC, 1], bf16, tag="ones_col")
            eye32 = pp.tile([T, T], bf16, tag="eye32")
            s_sb = pp.tile([1, NL], f32, tag="s_sb")
            warm = pp.tile([128, 512], bf16, tag="warm")
            dummy = pp.tile([1, 1], f32, tag="dummy")
            dummy_o = pp.tile([1, 1], f32, tag="dummy_o")

            # ---- PSUM tiles (6 banks: 1 + 1 + 2*2) ----
            ps_warm = psp.tile([128, 512], f32, tag="ps_warm")
            ps_s = psp.tile([1, NL], f32, tag="ps_s")
            ps_out = [psp.tile([T, D], f32, tag=f"ps_out{n}", name=f"ps_out{n}") for n in range(NL)]

            # ---- loads ----
            # w host-replicated bf16 over HWDGE (parallel to SWDGE img queue)
            nc.sync.dma_start(out=w_b, in_=wb[:, :])
            # img: one cast-DMA per (batch element, 98-row chunk) so each
            # chunk's score reduce can start as soon as its rows land
            img_src = [
                img[n, :, :].rearrange("(c p) d -> p c d", p=LC) for n in range(NL)
            ]
            for n in range(NL):
                for c in range(NC):
                    nc.gpsimd.dma_start(
                        out=img_t[n][:, c, :], in_=img_src[n][:, c, :]
                    )
            nc.gpsimd.dma_start(out=qes_t, in_=qes[:, :, :].transpose([1, 0, 2]))

            make_identity(nc, eye32)

            # ---- ACT exp-table preload + constants (DVE) ----
            nc.vector.memset(dummy, 0.0)
            nc.scalar.activation(dummy_o, dummy, mybir.ActivationFunctionType.Exp)
            nc.vector.memset(ones_col, 1.0)
            nc.vector.memset(warm, 0.0)

            # PE HAM warmup: ~8 bf16 N=512 matmuls ~= 3.4us busy at the cold
            # clock -> HAM flips to 8/8 before the real matmuls arrive.
            for i in range(8):
                nc.tensor.matmul(ps_warm, warm[:, 0:128], warm, start=True, stop=True)

            H = 512
            # ---- per-n pipeline with the S-fold trick ----
            # The output matmuls use UNNORMALIZED weights e = exp(s):
            #     psum = sum_l e[l]*img[l,:] + S_q*qes[t,:]
            # (identity scaled by S_q = bf16-quantized S), and the PSUM->SBUF
            # copy applies 1/S_q.  The qes term is exact (S_q * 1/S_q); the
            # attention term is normalized by S_q instead of S (0.4% on a
            # ~7%-magnitude term).  This takes S entirely off the critical
            # path: each chunk's matmuls fire right after its exp.
            for n in range(NL):
                for c in range(NC):
                    col = NC * n + c
                    prod = sp.tile([LC, D], bf16, tag="prod", name=f"prod{n}{c}")
                    nc.vector.affine_mul_reduce(
                        out=prod,
                        accum_out=s_all[:, col : col + 1],
                        in0=img_t[n][:, c, :],
                        in1=w_b[:LC, :],
                        scale=1.0,
                        bias=0.0,
                    )
                    # exp writes bf16 directly: the same quantized e feeds
                    # both the S sum and the weighted-sum matmuls, so the
                    # normalization is self-consistent.
                    nc.scalar.activation(
                        e_bf[:, col : col + 1],
                        s_all[:, col : col + 1],
                        mybir.ActivationFunctionType.Exp,
                    )
                    nc.tensor.matmul(
                        ps_s[0:1, n : n + 1],
                        e_bf[:, col : col + 1],
                        ones_col[:, :],
                        start=(c == 0),
                        stop=(c == NC - 1),
                    )

                # S path (parallel to the e@img matmuls, not on their chain):
                # S -> bf16-quantized S_q -> eyeS = eye*S_q, recip32 = 1/S_q
                nc.vector.tensor_copy(s_sb[:, n : n + 1], ps_s[0:1, n : n + 1])
                sq_bf = sp.tile([1, 1], bf16, tag="sq_bf", name=f"sq_bf{n}")
                sq_f = sp.tile([1, 1], f32, tag="sq_f", name=f"sq_f{n}")
                nc.gpsimd.tensor_copy(out=sq_bf, in_=s_sb[:, n : n + 1])
                nc.gpsimd.tensor_copy(out=sq_f, in_=sq_bf)
                s32 = sp.tile([T, 1], f32, tag="s32", name=f"s32_{n}")
                nc.gpsimd.partition_broadcast(s32, sq_f)
                eyeS = sp.tile([T, T], bf16, tag="eyeS", name=f"eyeS{n}")
                nc.gpsimd.tensor_scalar_mul(out=eyeS, in0=eye32, scalar1=s32)
                recip32 = sp.tile([T, 1], f32, tag="recip32", name=f"recip32_{n}")
                nc.vector.reciprocal(recip32, s32)

                # e @ img opens each accumulation group; eyeS@qes closes it
                for h in range(0, D, H):
                    for c in range(NC):
                        nc.tensor.matmul(
                            ps_out[n][:, h : h + H],
                            e_bf[:, NC * n + c : NC * n + c + 1].to_broadcast([LC, T]),
                            img_t[n][:, c, h : h + H],
                            start=(c == 0),
                            stop=False,
                        )
                    nc.tensor.matmul(
                        ps_out[n][:, h : h + H],
                        eyeS,
                        qes_t[:, n, h : h + H],
                        start=False,
                        stop=True,
                    )

                # PSUM -> SBUF with the 1/S_q scale, halves on alternating
                # engines, each DMA'd out immediately
                for h in range(0, D, H):
                    dst = out_sb[n * T : (n + 1) * T, h : h + H]
                    if h == 0 or n < NL - 1:
                        nc.scalar.activation(
                            dst,
                            ps_out[n][:, h : h + H],
                            mybir.ActivationFunctionType.Copy,
                            scale=recip32[:, :],
                        )
                    else:
                        nc.vector.tensor_scalar_mul(
                            out=dst,
                            in0=ps_out[n][:, h : h + H],
                            scalar1=recip32[:, :],
                        )
                    nc.sync.dma_start(
                        out=out[n * T : (n + 1) * T, h : h + H], in_=dst
                    )

    nc.compile()
    return nc


def _make_in_maps(inputs):
    """Shard the full inputs per core (data-parallel over N, 2 each)."""
    import ml_dtypes

    img_features = np.ascontiguousarray(inputs["img_features"], dtype=np.float32)
    qes_features = np.ascontiguousarray(inputs["qes_features"], dtype=np.float32)
    wb = np.ascontiguousarray(
        np.broadcast_to(
            np.asarray(inputs["w"], np.float32).astype(ml_dtypes.bfloat16)[None, :],
            (128, D),
        )
    )
    in_maps = []
    for c in range(N_CORES):
        sl = slice(NL * c, NL * (c + 1))
        in_maps.append({"img": img_features[sl], "qes": qes_features[sl], "wb": wb})
    return in_maps


def kernel(img_features, qes_features, w):
    from concourse.bass_utils import run_bass_kernel_spmd

    if "nc" not in _CACHE:
        _CACHE["nc"] = _build_nc()
    nc = _CACHE["nc"]

    in_maps = _make_in_maps(
        {"img_features": img_features, "qes_features": qes_features, "w": w}
    )
    res = run_bass_kernel_spmd(nc, in_maps, core_ids=list(range(N_CORES)))
    outs = [r["out"].reshape(NL, T, D) for r in res.results]
    return np.concatenate(outs, axis=0)


# revision 32
# speedup vs baseline: 1.0320x; 1.0229x over previous
"""Trainium2 Bass kernel for nn_Attention_layer_41429254537559.

Reference math:
    img_score = einsum('nld,d->nl', img, w)          # [N, L]
    q_score   = einsum('ntd,d->nt', qes, w)          # [N, T]
    logits    = q_score[:,:,None] + img_score[:,None,:]
    att       = softmax(logits, axis=2)              # over L
    out       = qes + einsum('ntl,nld->ntd', att, img)

Key simplification: q_score[n,t] is constant along the softmax axis (L), so it
cancels inside the softmax.  att[n,t,:] == softmax(img_score[n,:]) for every t:
    a[n,:]  = softmax(img @ w)        # [N, L]
    c[n,:]  = a[n,:] @ img[n]         # [N, D]
    out     = qes + c[:,None,:]

Distribution: data-parallel over N across 8 cores (2 batch elements per core).
No collectives needed.

Per-core dataflow (n_loc = 2, L = 196 = 2x98 chunks, D = 1024, T = 32):
  - img loaded as ONE SWDGE cast-DMA per batch element into a [98, 2, 1024]
    bf16 tile (l-rows split into two 98-row chunks across the free dim);
    qes likewise as one [32, 2, 1024] bf16 tile.  SWDGE descriptor emission
    costs ~1us of gpsimd time per dma_start, so fewer/larger DMAs win.
  - w arrives host-replicated as a [128, 1024] bf16 ExternalInput over the
    HWDGE queue — no on-chip broadcast, no cross-engine dependency chain.
  - s[l] = sum_d img[l,d]*w[d]: one DVE affine_mul_reduce per 98-row chunk
    (bf16 in, f32 accumulator column)
  - per n: e = exp(s) on ScalarE (|s| <~ 7, exp safe without max-shift),
    S_n = sum_l e[l] via tiny PE matmuls against a ones column,
    a = e * (1/S_n) (DVE reciprocal + gpsimd partition_broadcast), then
    materialized as [98, 32] bf16 lhsT tiles via tensor_scalar_mul on ones
  - out[n] = qes[n] + a @ img[n] as ONE accumulated PE matmul group per
    512-wide PSUM half: identity@qes first (inputs ready early), then the
    two a32 @ img-chunk matmuls; PSUM accumulates f32
  - PSUM -> SBUF f32 copy per half on alternating engines (ScalarE h0,
    VectorE h1), each half DMA'd out (HWDGE) immediately

8 bf16 warmup matmuls at t=0 keep the PE HAM clock warm; a dummy exp
preloads the ACT exp table during the DMA fill.
"""

import numpy as np

N_CORES = 8
N, L, D, T = 16, 196, 1024, 32
NL = N // N_CORES  # batch elements per core
NC = 2  # l-chunks per batch element
LC = L // NC  # 98 rows per chunk

_CACHE = {}


def _build_nc():
    import concourse.bass as bass
    import concourse.tile as tile
    from concourse import bacc, mybir
    from concourse.masks import make_identity

    f32 = mybir.dt.float32
    bf16 = mybir.dt.bfloat16
    nc = bacc.Bacc(None, target_bir_lowering=False)

    img = nc.dram_tensor("img", [NL, L, D], f32, kind="ExternalInput")
    qes = nc.dram_tensor("qes", [NL, T, D], f32, kind="ExternalInput")
    wb = nc.dram_tensor("wb", [128, D], bf16, kind="ExternalInput")
    out = nc.dram_tensor("out", [NL * T, D], f32, kind="ExternalOutput")

    with tile.TileContext(nc) as tc:
        with (
            tc.tile_pool(name="persist", bufs=1) as pp,
            tc.tile_pool(name="scratch", bufs=2) as sp,
            tc.tile_pool(name="psum", bufs=1, space="PSUM") as psp,
        ):
            # ---- persistent SBUF tiles ----
            w_b = pp.tile([128, D], bf16, tag="w_b")
            img_t = [pp.tile([LC, NC, D], bf16, tag=f"img{n}", name=f"img{n}") for n in range(NL)]
            qes_t = pp.tile([T, NL, D], bf16, tag="qes_t")
            out_sb = pp.tile([NL * T, D], f32, tag="out_sb")
            s_all = pp.tile([LC, NC * NL], f32, tag="s_all")
            e_bf = pp.tile([LC, NC * NL], bf16, tag="e_bf")
            ones_col = pp.tile([LC, 1], bf16, tag="ones_col")
            eye32 = pp.tile([T, T], bf16, tag="eye32")
            s_sb = pp.tile([1, NL], f32, tag="s_sb")
            warm = pp.tile([128, 512], bf16, tag="warm")
            dummy = pp.tile([1, 1], f32, tag="dummy")
            dummy_o = pp.tile([1, 1], f32, tag="dummy_o")

            # ---- PSUM tiles (6 banks: 1 + 1 + 2*2) ----
            ps_warm = psp.tile([128, 512], f32, tag="ps_warm")
            ps_s = psp.tile([1, NL], f32, tag="ps_s")
            ps_out = [psp.tile([T, D], f32, tag=f"ps_out{n}", name=f"ps_out{n}") for n in range(NL)]

            # ---- loads ----
            # w host-replicated bf16 over HWDGE (parallel to SWDGE img queue)
            nc.sync.dma_start(out=w_b, in_=wb[:, :])
            # img: one cast-DMA per (batch element, 98-row chunk) so each
            # chunk's score reduce can start as soon as its rows land
            img_src = [
                img[n, :, :].rearrange("(c p) d -> p c d", p=LC) for n in range(NL)
            ]
            for n in range(NL):
                for c in range(NC):
                    nc.gpsimd.dma_start(
                        out=img_t[n][:, c, :], in_=img_src[n][:, c, :]
                    )
            nc.gpsimd.dma_start(out=qes_t, in_=qes[:, :, :].transpose([1, 0, 2]))

            make_identity(nc, eye32)

            # ---- ACT exp-table preload + constants (DVE) ----
            nc.vector.memset(dummy, 0.0)
            nc.scalar.activation(dummy_o, dummy, mybir.ActivationFunctionType.Exp)
            nc.vector.memset(ones_col, 1.0)
            nc.vector.memset(warm, 0.0)

            # PE HAM warmup: ~8 bf16 N=512 matmuls ~= 3.4us busy at the cold
            # clock -> HAM flips to 8/8 before the real matmuls arrive.
            for i in range(8):
                nc.tensor.matmul(ps_warm, warm[:, 0:128], warm, start=True, stop=True)

            H = 512
            # ---- per-n pipeline with the S-fold trick ----
            # The output matmuls use UNNORMALIZED weights e = exp(s):
            #     psum = sum_l e[l]*img[l,:] + S_q*qes[t,:]
            # (identity scaled by S_q = bf16-quantized S), and the PSUM->SBUF
            # copy applies 1/S_q.  The qes term is exact (S_q * 1/S_q); the
            # attention term is normalized by S_q instead of S (0.4% on a
            # ~7%-magnitude term).  This takes S entirely off the critical
            # path: each chunk's matmuls fire right after its exp.
            for n in range(NL):
                for c in range(NC):
                    col = NC * n + c
                    prod = sp.tile([LC, D], bf16, tag="prod", name=f"prod{n}{c}")
                    nc.vector.affine_mul_reduce(
                        out=prod,
                        accum_out=s_all[:, col : col + 1],
                        in0=img_t[n][:, c, :],
                        in1=w_b[:LC, :],
                        scale=1.0,
                        bias=0.0,
                    )
                    # exp writes bf16 directly: the same quantized e feeds
                    # both the S sum and the weighted-sum matmuls, so the
                    # normalization is self-consistent.
                    nc.scalar.activation(
                        e_bf[:, col : col + 1],
                        s_all[:, col : col + 1],
                        mybir.ActivationFunctionType.Exp,
                    )
                    nc.tensor.matmul(
                        ps_s[0:1, n : n + 1],
                        e_bf[:, col : col + 1],
                        ones_col[:, :],
                        start=(c == 0),
                        stop=(c == NC - 1),
                    )

                # S path (parallel to the e@img matmuls, not on their chain):
                # S -> bf16-quantized S_q -> eyeS = eye*S_q, recip32 = 1/S_q
                sq_bf = sp.tile([1, 1], bf16, tag="sq_bf", name=f"sq_bf{n}")
                nc.vector.tensor_copy(sq_bf, ps_s[0:1, n : n + 1])
                s32b = sp.tile([T, 1], bf16, tag="s32b", name=f"s32b_{n}")
                nc.gpsimd.partition_broadcast(s32b, sq_bf)
                s32f = sp.tile([T, 1], f32, tag="s32f", name=f"s32f_{n}")
                nc.gpsimd.tensor_copy(out=s32f, in_=s32b)
                eyeS = sp.tile([T, T], bf16, tag="eyeS", name=f"eyeS{n}")
                nc.gpsimd.tensor_scalar_mul(out=eyeS, in0=eye32, scalar1=s32f)
                recip32 = sp.tile([T, 1], f32, tag="recip32", name=f"recip32_{n}")
                nc.vector.reciprocal(recip32, s32f)

                # e @ img opens each accumulation group; eyeS@qes closes it
                for h in range(0, D, H):
                    for c in range(NC):
                        nc.tensor.matmul(
                            ps_out[n][:, h : h + H],
                            e_bf[:, NC * n + c : NC * n + c + 1].to_broadcast([LC, T]),
                            img_t[n][:, c, h : h + H],
                            start=(c == 0),
                            stop=False,
                        )
                    nc.tensor.matmul(
                        ps_out[n][:, h : h + H],
                        eyeS,
                        qes_t[:, n, h : h + H],
                        start=False,
                        stop=True,
                    )

                # PSUM -> SBUF with the 1/S_q scale, halves on alternating
                # engines, each DMA'd out immediately
                for h in range(0, D, H):
                    dst = out_sb[n * T : (n + 1) * T, h : h + H]
                    if h == 0 or n < NL - 1:
                        nc.scalar.activation(
                            dst,
                            ps_out[n][:, h : h + H],
                            mybir.ActivationFunctionType.Copy,
                            scale=recip32[:, :],
                        )
                    else:
                        nc.vector.tensor_scalar_mul(
                            out=dst,
                            in0=ps_out[n][:, h : h + H],
                            scalar1=recip32[:, :],
                        )
                    nc.sync.dma_start(
                        out=out[n * T : (n + 1) * T, h : h + H], in_=dst
                    )

    nc.compile()
    return nc


def _make_in_maps(inputs):
    """Shard the full inputs per core (data-parallel over N, 2 each)."""
    import ml_dtypes

    img_features = np.ascontiguousarray(inputs["img_features"], dtype=np.float32)
    qes_features = np.ascontiguousarray(inputs["qes_features"], dtype=np.float32)
    wb = np.ascontiguousarray(
        np.broadcast_to(
            np.asarray(inputs["w"], np.float32).astype(ml_dtypes.bfloat16)[None, :],
            (128, D),
        )
    )
    in_maps = []
    for c in range(N_CORES):
        sl = slice(NL * c, NL * (c + 1))
        in_maps.append({"img": img_features[sl], "qes": qes_features[sl], "wb": wb})
    return in_maps


def kernel(img_features, qes_features, w):
    from concourse.bass_utils import run_bass_kernel_spmd

    if "nc" not in _CACHE:
        _CACHE["nc"] = _build_nc()
    nc = _CACHE["nc"]

    in_maps = _make_in_maps(
        {"img_features": img_features, "qes_features": qes_features, "w": w}
    )
    res = run_bass_kernel_spmd(nc, in_maps, core_ids=list(range(N_CORES)))
    outs = [r["out"].reshape(NL, T, D) for r in res.results]
    return np.concatenate(outs, axis=0)
